# revision 20
# baseline (speedup 1.0000x reference)
"""GroupViT cross-attention layer on 8 TRN2 NeuronCores.

Data-parallel over batch (2 per core). Feature-major layout on chip.
fp8e4+DoubleRow for K/V projections and probs@V; softmax exp split
between ACT (exact, free 1/2048 scale) and DVE (Schraudolph bit-trick
straight into e4m3); DVE reciprocal for denominators.

The two batches' attention phases are MERGED into one interleaved
B-phase so the tensor engine stays dense (HAM stays warm): per (hp,
so2) we emit scores/exp/ctx for both batches. Score tiles are
single-bank [128,512] (4 bufs) + 4 ctx accumulators = 8 PSUM banks.

Scale bookkeeping: ktch = 64*(k+bk) fp8, qt8 = 4*(q+bq) fp8 ->
score_psum = 2048*score_true -> exp scale 1/2048. v8 = 16*v fp8,
ones col at 64 -> ctx_psum = 16*ctx_unnorm, den row = sum(probs);
evict: rrec = 1/(16 den), ctxT = ctx_psum * bc(rrec).
"""

import numpy as np

B, T, S, D, H, HD, FF = 16, 512, 2048, 768, 12, 64, 3072
NCORES = 8
BPC = B // NCORES
P = 128
DC = D // P            # 6
SC = S // P            # 16
FFC = FF // P          # 24
FOG = 8                # fc stream groups per batch
FPG = FFC // FOG       # 3 fo-chunks per group
EPS = 1e-5
SCALE = HD ** -0.5
VPAD = 68              # v8 head stride (65 used + pad for DR step%16)

KSC = 64.0             # wk,bk host prescale
QSC = 4.0              # qt8 on-chip scale
SPS = KSC * QSC / SCALE   # score psum scale = 2048
EXPA = (8.0 / np.log(2.0)) / SPS   # DVE schraudolph mult
EXPC = 55.55                        # DVE schraudolph offset

_cached = {}


def _build(use_bv: bool):
    import concourse.bacc as bacc
    import concourse.tile as tile
    import concourse.mybir as mybir

    f32 = mybir.dt.float32
    f32r = mybir.dt.float32r
    bf16 = mybir.dt.bfloat16
    fp8 = mybir.dt.float8e4
    u8 = mybir.dt.uint8
    AF = mybir.ActivationFunctionType
    ALU = mybir.AluOpType
    DR = mybir.MatmulPerfMode.DoubleRow

    nc = bacc.Bacc("TRN2", target_bir_lowering=False, debug=False,
                   num_devices=NCORES)

    qbf_d = nc.dram_tensor("qbf", [BPC, D, T], bf16, kind="ExternalInput")
    k8_d = nc.dram_tensor("k8", [BPC, D, S], fp8, kind="ExternalInput")
    wq_d = nc.dram_tensor("wq_t", [D, D], bf16, kind="ExternalInput")
    wk_d = nc.dram_tensor("wk8", [D, D], fp8, kind="ExternalInput")
    wv_d = nc.dram_tensor("wv8", [D, D], fp8, kind="ExternalInput")
    wo_d = nc.dram_tensor("wo_t", [D, D], bf16, kind="ExternalInput")
    fc1_d = nc.dram_tensor("fc1_t", [D, FF], bf16, kind="ExternalInput")
    fc2_d = nc.dram_tensor("fc2_t", [FF, D], bf16, kind="ExternalInput")
    bq_d = nc.dram_tensor("bqv", [D], f32, kind="ExternalInput")
    bk_d = nc.dram_tensor("bk64", [D], f32, kind="ExternalInput")
    bv_d = nc.dram_tensor("bv16", [1, D], f32r, kind="ExternalInput")
    bo_d = nc.dram_tensor("bov", [D], f32, kind="ExternalInput")
    f1b_d = nc.dram_tensor("f1b", [FF], f32, kind="ExternalInput")
    f2b_d = nc.dram_tensor("f2b", [D], f32, kind="ExternalInput")
    ln2g_d = nc.dram_tensor("ln2g", [1, D], f32r, kind="ExternalInput")
    ln2bn_d = nc.dram_tensor("ln2bn", [1, D], f32r, kind="ExternalInput")
    lnpg_d = nc.dram_tensor("lnpg", [1, D], f32r, kind="ExternalInput")
    lnpbn_d = nc.dram_tensor("lnpbn", [1, D], f32r, kind="ExternalInput")
    ones_row_d = nc.dram_tensor("ones_row", [1, T], f32r, kind="ExternalInput")
    out_d = nc.dram_tensor("out", [BPC, D, T], f32, kind="ExternalOutput")

    from contextlib import ExitStack
    with tile.TileContext(nc) as tc, ExitStack() as est:
        def pool(**kw):
            return est.enter_context(tc.tile_pool(**kw))

        if True:
            small = pool(name="small", bufs=1)
            wts = pool(name="wts", bufs=1)
            qbfp = pool(name="qbfp", bufs=2)
            k8p = pool(name="k8p", bufs=2)
            qt8p = pool(name="qt8p", bufs=2)
            ktc = pool(name="ktc", bufs=2)
            v8p = pool(name="v8p", bufs=2)
            expp = pool(name="expp", bufs=2)
            ctxp = pool(name="ctxp", bufs=2)
            xtp = pool(name="xtp", bufs=2)
            htp = pool(name="htp", bufs=1)
            x2tp = pool(name="x2tp", bufs=1)
            fstream = pool(name="fstream", bufs=2)
            mchunkp = pool(name="mchunk", bufs=2)
            tmpp = pool(name="tmp", bufs=2)
            statp = pool(name="stat", bufs=1)
            evp = pool(name="evp", bufs=2)
            outp = pool(name="outp", bufs=1)

            # ---- batch inputs first on the DMA queues ----
            qbf = [None, None]
            k8 = [None, None]
            qbf[0] = qbfp.tile([P, DC, T], bf16, tag="qbf", name="qbf0")
            nc.sync.dma_start(qbf[0][:], qbf_d.ap()[0].rearrange(
                "(c p) t -> p c t", p=P))
            k8[0] = k8p.tile([P, DC, S], fp8, tag="k8", name="k80")
            nc.sync.dma_start(k8[0][:], k8_d.ap()[0].rearrange(
                "(c p) s -> p c s", p=P))
            qbf[1] = qbfp.tile([P, DC, T], bf16, tag="qbf", name="qbf1")
            nc.gpsimd.dma_start(qbf[1][:], qbf_d.ap()[1].rearrange(
                "(c p) t -> p c t", p=P))
            k8[1] = k8p.tile([P, DC, S], fp8, tag="k8", name="k81")
            nc.gpsimd.dma_start(k8[1][:], k8_d.ap()[1].rearrange(
                "(c p) s -> p c s", p=P))

            # ---- persistent weights ----
            wq_sb = wts.tile([P, DC, D], bf16, tag="wq")
            nc.sync.dma_start(wq_sb[:], wq_d.ap().rearrange(
                "(k p) o -> p k o", p=P))
            wv_sb = wts.tile([P, DC, D], fp8, tag="wv")
            nc.sync.dma_start(wv_sb[:], wv_d.ap().rearrange(
                "(k p) o -> p k o", p=P))
            wk_sb = wts.tile([P, DC, D], fp8, tag="wk")
            nc.sync.dma_start(wk_sb[:], wk_d.ap().rearrange(
                "(k p) o -> p k o", p=P))
            wo_sb = wts.tile([P, DC, D], bf16, tag="wo")
            nc.gpsimd.dma_start(wo_sb[:], wo_d.ap().rearrange(
                "(k p) o -> p k o", p=P))

            # ---- persistent smalls ----
            ones_col_bf = small.tile([P, 1], bf16, tag="ones_col_bf")
            nc.vector.memset(ones_col_bf[:], 1.0)
            ones_row = small.tile([1, T], f32r, tag="ones_row")
            nc.sync.dma_start(ones_row[:], ones_row_d.ap())
            ones64_bf = small.tile([1, HD], bf16, tag="ones64")
            nc.vector.memset(ones64_bf[:], 1.0)
            eps_t = small.tile([1, 1], f32, tag="eps")
            nc.vector.memset(eps_t[:], EPS)

            ln2g = small.tile([1, D], f32r, tag="ln2g")
            nc.sync.dma_start(ln2g[:], ln2g_d.ap())
            ln2bn = small.tile([1, D], f32r, tag="ln2bn")
            nc.sync.dma_start(ln2bn[:], ln2bn_d.ap())
            lnpg = small.tile([1, D], f32r, tag="lnpg")
            nc.sync.dma_start(lnpg[:], lnpg_d.ap())
            lnpbn = small.tile([1, D], f32r, tag="lnpbn")
            nc.sync.dma_start(lnpbn[:], lnpbn_d.ap())

            bq_pc = small.tile([P, DC], f32, tag="bq_pc")
            nc.sync.dma_start(bq_pc[:], bq_d.ap().rearrange("(c p) -> p c", p=P))
            bk_pc = small.tile([P, DC], f32, tag="bk_pc")
            nc.sync.dma_start(bk_pc[:], bk_d.ap().rearrange("(c p) -> p c", p=P))
            bo_pc = small.tile([P, DC], f32, tag="bo_pc")
            nc.sync.dma_start(bo_pc[:], bo_d.ap().rearrange("(c p) -> p c", p=P))
            f1b_pc = small.tile([P, FFC], f32, tag="f1b_pc")
            nc.sync.dma_start(f1b_pc[:], f1b_d.ap().rearrange("(c p) -> p c", p=P))
            f2b_pc = small.tile([P, DC], f32, tag="f2b_pc")
            nc.sync.dma_start(f2b_pc[:], f2b_d.ap().rearrange("(c p) -> p c", p=P))

            bv_bc = None
            if use_bv:
                bv_row = small.tile([1, D], f32r, tag="bv_row")
                nc.sync.dma_start(bv_row[:], bv_d.ap())

            # =========== phase A (both batches) ===========
            qt8 = [None, None]
            v8 = [None, None]
            with tc.tile_pool(name="psAA", bufs=3, space="PSUM") as psP:
                if use_bv:
                    bv_bc = small.tile([P, D], f32, tag="bv_bc")
                    for half in range(2):
                        ps_bv = psP.tile([P, 512], f32, tag="psP")
                        nc.tensor.matmul(
                            ps_bv[:, 0:384], ones_row[:, 0:P],
                            bv_row[:, half * 384:(half + 1) * 384],
                            start=True, stop=True)
                        nc.vector.tensor_copy(
                            bv_bc[:, half * 384:(half + 1) * 384],
                            ps_bv[:, 0:384])
                for b in range(BPC):
                    qt8[b] = qt8p.tile([P, DC, T], fp8, tag="qt8",
                                       name=f"qt8_{b}")
                    for mo in range(DC):
                        ps = psP.tile([P, 512], f32, tag="psP")
                        for ki in range(DC):
                            nc.tensor.matmul(
                                ps[:], wq_sb[:, ki, mo * P:(mo + 1) * P],
                                qbf[b][:, ki, :],
                                start=(ki == 0), stop=(ki == DC - 1))
                        nc.vector.tensor_scalar(
                            qt8[b][:, mo, :], ps[:], bq_pc[:, mo:mo + 1], QSC,
                            op0=ALU.add, op1=ALU.mult)
                    v8[b] = v8p.tile([P, SC, H, VPAD], fp8, tag="v8",
                                     name=f"v8_{b}")
                    nc.vector.memset(v8[b][:, :, :, HD:VPAD], 0.0)
                    nc.vector.memset(v8[b][:, :, :, HD:HD + 1], 1.0)
                    for so in range(SC):
                        for half in range(2):
                            ps = psP.tile([P, 512], f32, tag="psP")
                            for k2 in range(DC // 2):
                                nc.tensor.matmul(
                                    ps[:, 0:384],
                                    k8[b][:, 2 * k2:2 * k2 + 2,
                                          so * P:(so + 1) * P],
                                    wv_sb[:, 2 * k2:2 * k2 + 2,
                                          half * 384:(half + 1) * 384],
                                    start=(k2 == 0), stop=(k2 == DC // 2 - 1),
                                    perf_mode=DR)
                            dstv = v8[b][:, so, half * 6:(half + 1) * 6, 0:HD]
                            if use_bv:
                                nc.vector.scalar_tensor_tensor(
                                    dstv, ps[:, 0:384], 0.25,
                                    bv_bc[:, half * 384:(half + 1) * 384],
                                    op0=ALU.mult, op1=ALU.add)
                            elif so % 2 == 0:
                                nc.scalar.mul(dstv, ps[:, 0:384], 0.25)
                            else:
                                nc.vector.tensor_scalar_mul(
                                    dstv, ps[:, 0:384], 0.25)

            # =========== merged attention phase (both batches) ===========
            ctxT = [None, None]
            ctxT[0] = ctxp.tile([P, DC, T], bf16, tag="ctxT", name="ctxT0")
            ctxT[1] = ctxp.tile([P, DC, T], bf16, tag="ctxT", name="ctxT1")

            def attn_kproj(b, hp, psSC):
                ktch = ktc.tile([P, S], fp8, tag="ktc", name=f"kt{b}")
                for no in range(4):
                    pst = psSC.tile([P, 2, 512], f32, tag="psSC")
                    ps = pst[:, 0, :]
                    for k2 in range(DC // 2):
                        nc.tensor.matmul(
                            ps,
                            wk_sb[:, 2 * k2:2 * k2 + 2, hp * P:(hp + 1) * P],
                            k8[b][:, 2 * k2:2 * k2 + 2, no * T:(no + 1) * T],
                            start=(k2 == 0), stop=(k2 == DC // 2 - 1),
                            perf_mode=DR)
                    if no % 2 == 0:
                        nc.scalar.activation(
                            ktch[:, no * T:(no + 1) * T], ps,
                            AF.Identity, bias=bk_pc[:, hp:hp + 1])
                    else:
                        nc.vector.tensor_scalar_add(
                            ktch[:, no * T:(no + 1) * T], ps,
                            bk_pc[:, hp:hp + 1])
                return ktch

            def attn_scores(b, hp, so2, ktch, ps_ctx, psSC):
                ex = expp.tile([P, 2, 2, 512], fp8, tag="exp", name=f"ex{b}")
                for j in range(2):
                    so = so2 + j
                    scj = psSC.tile([P, 2, 512], f32, tag="psSC")
                    for hh in range(2):
                        base = hh * HD
                        nc.tensor.matmul(
                            scj[:, hh, :],
                            ktch[base:base + HD, so * P:(so + 1) * P],
                            qt8[b][base:base + HD, hp, :],
                            start=True, stop=True)
                    # exp of both heads' chunk j: ACT j0, DVE j1
                    if j == 0:
                        nc.scalar.activation(ex[:, :, 0, :], scj[:],
                                             AF.Exp, scale=1.0 / SPS)
                    else:
                        nc.vector.tensor_scalar(
                            ex[:, :, 1, :].bitcast(u8), scj[:],
                            EXPA, EXPC, op0=ALU.mult, op1=ALU.add)
                for hh in range(2):
                    h = 2 * hp + hh
                    nc.tensor.matmul(
                        ps_ctx[hh][:], v8[b][:, so2:so2 + 2, h, :],
                        ex[:, hh, :, :], start=(so2 == 0),
                        stop=(so2 == SC - 2), perf_mode=DR)

            def attn_evict(b, hp, hh, ps_ctx, psSC):
                base = hh * HD
                rden_f = evp.tile([1, T], f32, tag="rden_f")
                nc.vector.tensor_scalar_mul(
                    rden_f[:], ps_ctx[hh][HD:HD + 1, :], 16.0)
                rrec = evp.tile([1, T], f32, tag="rrec")
                nc.vector.reciprocal_approx_fast(out=rrec[:], in_=rden_f[:])
                rden_bf = evp.tile([1, T], bf16, tag="rden_bf")
                nc.scalar.copy(rden_bf[:], rrec[:])
                ps_bct = psSC.tile([P, 2, 512], f32, tag="psSC")
                ps_bc = ps_bct[:, 0, :]
                nc.tensor.matmul(ps_bc[0:HD, :], ones64_bf[:],
                                 rden_bf[:], start=True, stop=True)
                bc_sb = evp.tile([HD, T], bf16, tag="bc_sb")
                nc.scalar.copy(bc_sb[:], ps_bc[0:HD, :])
                nc.vector.tensor_tensor(
                    ctxT[b][base:base + HD, hp, :],
                    ps_ctx[hh][0:HD, :], bc_sb[:], ALU.mult)

            with (
                tc.tile_pool(name="psSC", bufs=2, space="PSUM") as psSC,
                tc.tile_pool(name="psCTX", bufs=4, space="PSUM") as psCTX,
            ):
                for hp in range(DC):
                    ktch = [attn_kproj(b, hp, psSC) for b in range(BPC)]
                    ps_ctx = [[psCTX.tile([VPAD, T], f32, tag="psCTX",
                                          name=f"ps_ctx{b}_{i}")
                               for i in range(2)] for b in range(BPC)]
                    for so2 in range(0, SC, 2):
                        for b in range(BPC):
                            attn_scores(b, hp, so2, ktch[b], ps_ctx[b], psSC)
                    for b in range(BPC):
                        for hh in range(2):
                            attn_evict(b, hp, hh, ps_ctx[b], psSC)

            # =========== per-batch tail helpers ===========

            def phase_C(b, xT, psP):
                for mo in range(DC):
                    ps = psP.tile([P, 512], f32, tag="psP")
                    for ki in range(DC):
                        nc.tensor.matmul(ps[:], wo_sb[:, ki, mo * P:(mo + 1) * P],
                                         ctxT[b][:, ki, :],
                                         start=(ki == 0), stop=(ki == DC - 1))
                    nc.vector.scalar_tensor_tensor(
                        xT[:, mo, :], ps[:], bo_pc[:, mo:mo + 1],
                        qbf[b][:, mo, :], op0=ALU.add, op1=ALU.add)

            def ln_pass(b, tag, xsrc, g_row, bn_row, dst=None, store=False):
                with (
                    tc.tile_pool(name=f"psST{tag}{b}", bufs=1,
                                 space="PSUM") as ps_st,
                    tc.tile_pool(name=f"psLB{tag}{b}", bufs=2,
                                 space="PSUM") as ps_bc,
                ):
                    psum_mu = ps_st.tile([1, T], f32, tag="st_mu")
                    psum_sq = ps_st.tile([1, T], f32, tag="st_sq")
                    for c in range(DC):
                        nc.tensor.matmul(psum_mu[:], ones_col_bf[:],
                                         xsrc[:, c, :],
                                         start=(c == 0), stop=(c == DC - 1))
                    sqt = []
                    for c in range(DC):
                        sq = tmpp.tile([P, T], bf16, tag="lnsq")
                        nc.vector.tensor_mul(sq[:], xsrc[:, c, :], xsrc[:, c, :])
                        sqt.append(sq)
                    for c in range(DC):
                        nc.tensor.matmul(psum_sq[:], ones_col_bf[:], sqt[c][:],
                                         start=(c == 0), stop=(c == DC - 1))
                    mu_f = statp.tile([1, T], f32, tag="ln_mu")
                    nc.vector.tensor_scalar_mul(mu_f[:], psum_mu[:], 1.0 / D)
                    mu2_f = statp.tile([1, T], f32, tag="ln_mu2")
                    nc.vector.tensor_tensor(mu2_f[:], mu_f[:], mu_f[:], ALU.mult)
                    var_f = statp.tile([1, T], f32, tag="ln_var")
                    nc.vector.scalar_tensor_tensor(
                        var_f[:], psum_sq[:], 1.0 / D, mu2_f[:],
                        op0=ALU.mult, op1=ALU.subtract)
                    rs_f = statp.tile([1, T], f32, tag="ln_rs")
                    nc.scalar.activation(rs_f[:], var_f[:],
                                         AF.Abs_reciprocal_sqrt, bias=eps_t[:])
                    rs_r = statp.tile([1, T], f32r, tag="ln_rs_r")
                    nc.vector.tensor_copy(rs_r[:], rs_f[:])
                    mrs_r = statp.tile([1, T], f32r, tag="ln_mrs_r")
                    nc.vector.tensor_tensor(mrs_r[:], mu_f[:], rs_f[:],
                                            ALU.mult)
                    for c in range(DC):
                        bcA = ps_bc.tile([P, T], f32, tag="ln_bcA")
                        bcB = ps_bc.tile([P, T], f32, tag="ln_bcB")
                        gsl = g_row[:, c * P:(c + 1) * P]
                        bsl = bn_row[:, c * P:(c + 1) * P]
                        nc.tensor.matmul(bcA[:], gsl, rs_r[:],
                                         start=True, stop=True)
                        nc.tensor.matmul(bcB[:], gsl, mrs_r[:],
                                         start=True, stop=False)
                        nc.tensor.matmul(bcB[:], bsl, ones_row[:],
                                         start=False, stop=True)
                        tmp = tmpp.tile([P, T], f32, tag="ln_tmp")
                        nc.vector.tensor_tensor(tmp[:], xsrc[:, c, :], bcA[:],
                                                ALU.mult)
                        if store:
                            oc = outp.tile([P, T], f32, tag="outT")
                            nc.vector.tensor_tensor(oc[:], tmp[:], bcB[:],
                                                    ALU.subtract)
                            nc.sync.dma_start(
                                out_d.ap()[b][c * P:(c + 1) * P, :], oc[:])
                        else:
                            nc.vector.tensor_tensor(dst[:, c, :], tmp[:],
                                                    bcB[:], ALU.subtract)

            def phase_E(b, xT, hT, x2T):
                with (
                    tc.tile_pool(name=f"psE1{b}", bufs=2, space="PSUM") as psE1,
                    tc.tile_pool(name=f"psF2{b}", bufs=6, space="PSUM") as psF2,
                ):
                    ps_f2 = [psF2.tile([P, T], f32, tag="psF2", name=f"psf2_{i}")
                             for i in range(DC)]
                    for g in range(FOG):
                        f1g = fstream.tile([P, DC, FPG * P], bf16, tag="f1g")
                        nc.sync.dma_start(f1g[:], fc1_d.ap().rearrange(
                            "(k p) f -> p k f", p=P)[:, :, g * FPG * P:
                                                     (g + 1) * FPG * P])
                        f2g = fstream.tile([P, FPG, D], bf16, tag="f2g")
                        nc.gpsimd.dma_start(f2g[:], fc2_d.ap().rearrange(
                            "(ko p) o -> p ko o", p=P)[:, g * FPG:(g + 1) * FPG, :])
                        for j in range(FPG):
                            fo = g * FPG + j
                            ps1 = psE1.tile([P, 512], f32, tag="psE1")
                            for ki in range(DC):
                                nc.tensor.matmul(
                                    ps1[:], f1g[:, ki, j * P:(j + 1) * P],
                                    hT[:, ki, :],
                                    start=(ki == 0), stop=(ki == DC - 1))
                            mch = mchunkp.tile([P, T], bf16, tag="mch")
                            nc.scalar.activation(mch[:], ps1[:], AF.Gelu,
                                                 bias=f1b_pc[:, fo:fo + 1])
                            for mo in range(DC):
                                nc.tensor.matmul(
                                    ps_f2[mo][:], f2g[:, j, mo * P:(mo + 1) * P],
                                    mch[:],
                                    start=(fo == 0), stop=(fo == FFC - 1))
                    for mo in range(DC):
                        nc.vector.scalar_tensor_tensor(
                            x2T[:, mo, :], ps_f2[mo][:], f2b_pc[:, mo:mo + 1],
                            xT[:, mo, :], op0=ALU.add, op1=ALU.add)

            # =========== tail emission ===========
            xT = [None, None]
            hT = [None, None]
            x2T = [None, None]
            with tc.tile_pool(name="psC", bufs=2, space="PSUM") as psP2:
                for b in range(BPC):
                    xT[b] = xtp.tile([P, DC, T], bf16, tag="xT", name=f"xT{b}")
                    phase_C(b, xT[b], psP2)
            hT[0] = htp.tile([P, DC, T], bf16, tag="hT", name="hT0")
            ln_pass(0, "2", xT[0], ln2g, ln2bn, dst=hT[0])
            x2T[0] = x2tp.tile([P, DC, T], bf16, tag="x2T", name="x2T0")
            phase_E(0, xT[0], hT[0], x2T[0])
            hT[1] = htp.tile([P, DC, T], bf16, tag="hT", name="hT1")
            ln_pass(1, "2", xT[1], ln2g, ln2bn, dst=hT[1])
            ln_pass(0, "p", x2T[0], lnpg, lnpbn, store=True)
            x2T[1] = x2tp.tile([P, DC, T], bf16, tag="x2T", name="x2T1")
            phase_E(1, xT[1], hT[1], x2T[1])
            ln_pass(1, "p", x2T[1], lnpg, lnpbn, store=True)

    nc.compile()
    return nc


def _get_nc(use_bv: bool):
    key = ("nc", use_bv)
    if key not in _cached:
        _cached[key] = _build(use_bv)
    return _cached[key]


def kernel(query, key, wq, bq, wk, bk, wv, bv, wo, bo,
           ln2_g, ln2_b, fc1_w, fc1_b, fc2_w, fc2_b, lnp_g, lnp_b):
    import ml_dtypes
    from concourse.bass_utils import run_bass_kernel_spmd

    f = np.float32
    bf = ml_dtypes.bfloat16
    f8 = ml_dtypes.float8_e4m3fn
    c = np.ascontiguousarray
    query = np.asarray(query, f)
    key = np.asarray(key, f)
    use_bv = bool(np.any(np.asarray(bv)))
    nc = _get_nc(use_bv)

    shared = {
        "wq_t": c(np.asarray(wq, f).T.astype(bf)),
        "wk8": c((np.asarray(wk, f).T * np.float32(KSC)).astype(f8)),
        "wv8": c((np.asarray(wv, f).T * np.float32(KSC)).astype(f8)),
        "wo_t": c(np.asarray(wo, f).T.astype(bf)),
        "fc1_t": c(np.asarray(fc1_w, f).T.astype(bf)),
        "fc2_t": c(np.asarray(fc2_w, f).T.astype(bf)),
        "bqv": c(np.asarray(bq, f)),
        "bk64": c(np.asarray(bk, f) * np.float32(KSC)),
        "bv16": c(np.asarray(bv, f).reshape(1, D) * np.float32(16.0)),
        "bov": c(np.asarray(bo, f)),
        "f1b": c(np.asarray(fc1_b, f)),
        "f2b": c(np.asarray(fc2_b, f)),
        "ln2g": c(np.asarray(ln2_g, f).reshape(1, D)),
        "ln2bn": c(-np.asarray(ln2_b, f).reshape(1, D)),
        "lnpg": c(np.asarray(lnp_g, f).reshape(1, D)),
        "lnpbn": c(-np.asarray(lnp_b, f).reshape(1, D)),
        "ones_row": np.ones((1, T), f),
    }
    in_maps = []
    for core in range(NCORES):
        sl = slice(core * BPC, (core + 1) * BPC)
        m = dict(shared)
        m["qbf"] = c(query[sl].transpose(0, 2, 1).astype(bf))
        m["k8"] = c(key[sl].transpose(0, 2, 1).astype(f8))
        in_maps.append(m)

    res = run_bass_kernel_spmd(nc, in_maps, core_ids=list(range(NCORES)))
    kernel._last_result = res
    out = np.concatenate([r["out"] for r in res.results], axis=0)
    return c(out.transpose(0, 2, 1))


# revision 22
# speedup vs baseline: 1.4152x; 1.4152x over previous
"""GroupViT cross-attention layer on 8 TRN2 NeuronCores.

Data-parallel over batch (2 per core). Feature-major layout on chip.
fp8e4+DoubleRow for K/V projections and probs@V; softmax exp split
between ACT (exact, free 1/2048 scale) and DVE (Schraudolph bit-trick
straight into e4m3); DVE reciprocal for denominators.

The two batches' attention phases are MERGED into one interleaved
B-phase so the tensor engine stays dense (HAM stays warm): per (hp,
so2) we emit scores/exp/ctx for both batches. Score tiles are
single-bank [128,512] (4 bufs) + 4 ctx accumulators = 8 PSUM banks.

Scale bookkeeping: ktch = 64*(k+bk) fp8, qt8 = 4*(q+bq) fp8 ->
score_psum = 2048*score_true -> exp scale 1/2048. v8 = 16*v fp8,
ones col at 64 -> ctx_psum = 16*ctx_unnorm, den row = sum(probs);
evict: rrec = 1/(16 den), ctxT = ctx_psum * bc(rrec).
"""

import numpy as np

B, T, S, D, H, HD, FF = 16, 512, 2048, 768, 12, 64, 3072
NCORES = 8
BPC = B // NCORES
P = 128
DC = D // P            # 6
SC = S // P            # 16
FFC = FF // P          # 24
FOG = 8                # fc stream groups per batch
FPG = FFC // FOG       # 3 fo-chunks per group
EPS = 1e-5
SCALE = HD ** -0.5
VPAD = 68              # v8 head stride (65 used + pad for DR step%16)

KSC = 64.0             # wk,bk host prescale
QSC = 4.0              # qt8 on-chip scale
SPS = KSC * QSC / SCALE   # score psum scale = 2048
EXPA = (8.0 / np.log(2.0)) / SPS   # DVE schraudolph mult
EXPC = 55.55                        # DVE schraudolph offset

_cached = {}


def _build(use_bv: bool):
    import concourse.bacc as bacc
    import concourse.tile as tile
    import concourse.mybir as mybir

    f32 = mybir.dt.float32
    f32r = mybir.dt.float32r
    bf16 = mybir.dt.bfloat16
    fp8 = mybir.dt.float8e4
    u8 = mybir.dt.uint8
    AF = mybir.ActivationFunctionType
    ALU = mybir.AluOpType
    DR = mybir.MatmulPerfMode.DoubleRow

    nc = bacc.Bacc("TRN2", target_bir_lowering=False, debug=False,
                   num_devices=NCORES)

    qbf_d = nc.dram_tensor("qbf", [BPC, D, T], bf16, kind="ExternalInput")
    k8_d = nc.dram_tensor("k8", [BPC, D, S], fp8, kind="ExternalInput")
    wq_d = nc.dram_tensor("wq_t", [D, D], bf16, kind="ExternalInput")
    wk_d = nc.dram_tensor("wk8", [D, D], fp8, kind="ExternalInput")
    wv_d = nc.dram_tensor("wv8", [D, D], fp8, kind="ExternalInput")
    wo_d = nc.dram_tensor("wo_t", [D, D], bf16, kind="ExternalInput")
    fc1_d = nc.dram_tensor("fc1_t", [D, FF], bf16, kind="ExternalInput")
    fc2_d = nc.dram_tensor("fc2_t", [FF, D], bf16, kind="ExternalInput")
    bq_d = nc.dram_tensor("bqv", [D], f32, kind="ExternalInput")
    bk_d = nc.dram_tensor("bk64", [D], f32, kind="ExternalInput")
    bv_d = nc.dram_tensor("bv16", [1, D], f32r, kind="ExternalInput")
    bo_d = nc.dram_tensor("bov", [D], f32, kind="ExternalInput")
    f1b_d = nc.dram_tensor("f1b", [FF], f32, kind="ExternalInput")
    f2b_d = nc.dram_tensor("f2b", [D], f32, kind="ExternalInput")
    ln2g_d = nc.dram_tensor("ln2g", [1, D], f32r, kind="ExternalInput")
    ln2bn_d = nc.dram_tensor("ln2bn", [1, D], f32r, kind="ExternalInput")
    lnpg_d = nc.dram_tensor("lnpg", [1, D], f32r, kind="ExternalInput")
    lnpbn_d = nc.dram_tensor("lnpbn", [1, D], f32r, kind="ExternalInput")
    ones_row_d = nc.dram_tensor("ones_row", [1, T], f32r, kind="ExternalInput")
    out_d = nc.dram_tensor("out", [BPC, D, T], f32, kind="ExternalOutput")

    from contextlib import ExitStack
    with tile.TileContext(nc) as tc, ExitStack() as est:
        def pool(**kw):
            return est.enter_context(tc.tile_pool(**kw))

        if True:
            small = pool(name="small", bufs=1)
            wts = pool(name="wts", bufs=1)
            qbfp = pool(name="qbfp", bufs=2)
            k8p = pool(name="k8p", bufs=2)
            qt8p = pool(name="qt8p", bufs=2)
            ktc = pool(name="ktc", bufs=2)
            v8p = pool(name="v8p", bufs=2)
            expp = pool(name="expp", bufs=2)
            ctxp = pool(name="ctxp", bufs=2)
            xtp = pool(name="xtp", bufs=2)
            htp = pool(name="htp", bufs=1)
            x2tp = pool(name="x2tp", bufs=1)
            fstream = pool(name="fstream", bufs=2)
            mchunkp = pool(name="mchunk", bufs=2)
            tmpp = pool(name="tmp", bufs=2)
            statp = pool(name="stat", bufs=1)
            evp = pool(name="evp", bufs=2)
            outp = pool(name="outp", bufs=1)

            # ---- batch inputs first on the DMA queues ----
            qbf = [None, None]
            k8 = [None, None]
            qbf[0] = qbfp.tile([P, DC, T], bf16, tag="qbf", name="qbf0")
            nc.sync.dma_start(qbf[0][:], qbf_d.ap()[0].rearrange(
                "(c p) t -> p c t", p=P))
            k8[0] = k8p.tile([P, DC, S], fp8, tag="k8", name="k80")
            nc.sync.dma_start(k8[0][:], k8_d.ap()[0].rearrange(
                "(c p) s -> p c s", p=P))
            qbf[1] = qbfp.tile([P, DC, T], bf16, tag="qbf", name="qbf1")
            nc.gpsimd.dma_start(qbf[1][:], qbf_d.ap()[1].rearrange(
                "(c p) t -> p c t", p=P))
            k8[1] = k8p.tile([P, DC, S], fp8, tag="k8", name="k81")
            nc.gpsimd.dma_start(k8[1][:], k8_d.ap()[1].rearrange(
                "(c p) s -> p c s", p=P))

            # ---- persistent weights ----
            wq_sb = wts.tile([P, DC, D], bf16, tag="wq")
            nc.sync.dma_start(wq_sb[:], wq_d.ap().rearrange(
                "(k p) o -> p k o", p=P))
            wv_sb = wts.tile([P, DC, D], fp8, tag="wv")
            nc.sync.dma_start(wv_sb[:], wv_d.ap().rearrange(
                "(k p) o -> p k o", p=P))
            wk_sb = wts.tile([P, DC, D], fp8, tag="wk")
            nc.sync.dma_start(wk_sb[:], wk_d.ap().rearrange(
                "(k p) o -> p k o", p=P))
            wo_sb = wts.tile([P, DC, D], bf16, tag="wo")
            nc.gpsimd.dma_start(wo_sb[:], wo_d.ap().rearrange(
                "(k p) o -> p k o", p=P))

            # ---- persistent smalls ----
            ones_col_bf = small.tile([P, 1], bf16, tag="ones_col_bf")
            nc.vector.memset(ones_col_bf[:], 1.0)
            ones_row = small.tile([1, T], f32r, tag="ones_row")
            nc.sync.dma_start(ones_row[:], ones_row_d.ap())
            ones64_bf = small.tile([1, HD], bf16, tag="ones64")
            nc.vector.memset(ones64_bf[:], 1.0)
            eps_t = small.tile([1, 1], f32, tag="eps")
            nc.vector.memset(eps_t[:], EPS)

            ln2g = small.tile([1, D], f32r, tag="ln2g")
            nc.sync.dma_start(ln2g[:], ln2g_d.ap())
            ln2bn = small.tile([1, D], f32r, tag="ln2bn")
            nc.sync.dma_start(ln2bn[:], ln2bn_d.ap())
            lnpg = small.tile([1, D], f32r, tag="lnpg")
            nc.sync.dma_start(lnpg[:], lnpg_d.ap())
            lnpbn = small.tile([1, D], f32r, tag="lnpbn")
            nc.sync.dma_start(lnpbn[:], lnpbn_d.ap())

            bq_pc = small.tile([P, DC], f32, tag="bq_pc")
            nc.sync.dma_start(bq_pc[:], bq_d.ap().rearrange("(c p) -> p c", p=P))
            bk_pc = small.tile([P, DC], f32, tag="bk_pc")
            nc.sync.dma_start(bk_pc[:], bk_d.ap().rearrange("(c p) -> p c", p=P))
            bo_pc = small.tile([P, DC], f32, tag="bo_pc")
            nc.sync.dma_start(bo_pc[:], bo_d.ap().rearrange("(c p) -> p c", p=P))
            f1b_pc = small.tile([P, FFC], f32, tag="f1b_pc")
            nc.sync.dma_start(f1b_pc[:], f1b_d.ap().rearrange("(c p) -> p c", p=P))
            f2b_pc = small.tile([P, DC], f32, tag="f2b_pc")
            nc.sync.dma_start(f2b_pc[:], f2b_d.ap().rearrange("(c p) -> p c", p=P))

            bv_bc = None
            if use_bv:
                bv_row = small.tile([1, D], f32r, tag="bv_row")
                nc.sync.dma_start(bv_row[:], bv_d.ap())

            # =========== phase A (both batches) ===========
            qt8 = [None, None]
            v8 = [None, None]
            with tc.tile_pool(name="psAA", bufs=3, space="PSUM") as psP:
                if use_bv:
                    bv_bc = small.tile([P, D], f32, tag="bv_bc")
                    for half in range(2):
                        ps_bv = psP.tile([P, 512], f32, tag="psP")
                        nc.tensor.matmul(
                            ps_bv[:, 0:384], ones_row[:, 0:P],
                            bv_row[:, half * 384:(half + 1) * 384],
                            start=True, stop=True)
                        nc.vector.tensor_copy(
                            bv_bc[:, half * 384:(half + 1) * 384],
                            ps_bv[:, 0:384])
                for b in range(BPC):
                    qt8[b] = qt8p.tile([P, DC, T], fp8, tag="qt8",
                                       name=f"qt8_{b}")
                    for mo in range(DC):
                        ps = psP.tile([P, 512], f32, tag="psP")
                        for ki in range(DC):
                            nc.tensor.matmul(
                                ps[:], wq_sb[:, ki, mo * P:(mo + 1) * P],
                                qbf[b][:, ki, :],
                                start=(ki == 0), stop=(ki == DC - 1))
                        nc.vector.tensor_scalar(
                            qt8[b][:, mo, :], ps[:], bq_pc[:, mo:mo + 1], QSC,
                            op0=ALU.add, op1=ALU.mult)
                    v8[b] = v8p.tile([P, SC, H, VPAD], fp8, tag="v8",
                                     name=f"v8_{b}")
                    nc.vector.memset(v8[b][:, :, :, HD:VPAD], 0.0)
                    nc.vector.memset(v8[b][:, :, :, HD:HD + 1], 1.0)
                    for so in range(SC):
                        for half in range(2):
                            ps = psP.tile([P, 512], f32, tag="psP")
                            for k2 in range(DC // 2):
                                nc.tensor.matmul(
                                    ps[:, 0:384],
                                    k8[b][:, 2 * k2:2 * k2 + 2,
                                          so * P:(so + 1) * P],
                                    wv_sb[:, 2 * k2:2 * k2 + 2,
                                          half * 384:(half + 1) * 384],
                                    start=(k2 == 0), stop=(k2 == DC // 2 - 1),
                                    perf_mode=DR)
                            dstv = v8[b][:, so, half * 6:(half + 1) * 6, 0:HD]
                            if use_bv:
                                nc.vector.scalar_tensor_tensor(
                                    dstv, ps[:, 0:384], 0.25,
                                    bv_bc[:, half * 384:(half + 1) * 384],
                                    op0=ALU.mult, op1=ALU.add)
                            else:
                                nc.scalar.mul(dstv, ps[:, 0:384], 0.25)

            # =========== merged attention phase (both batches) ===========
            ctxT = [None, None]
            ctxT[0] = ctxp.tile([P, DC, T], bf16, tag="ctxT", name="ctxT0")
            ctxT[1] = ctxp.tile([P, DC, T], bf16, tag="ctxT", name="ctxT1")

            def attn_kproj(b, hp, psSC):
                ktch = ktc.tile([P, S], fp8, tag="ktc", name=f"kt{b}")
                for no in range(4):
                    pst = psSC.tile([P, 512], f32, tag="psSC")
                    ps = pst[:]
                    for k2 in range(DC // 2):
                        nc.tensor.matmul(
                            ps,
                            wk_sb[:, 2 * k2:2 * k2 + 2, hp * P:(hp + 1) * P],
                            k8[b][:, 2 * k2:2 * k2 + 2, no * T:(no + 1) * T],
                            start=(k2 == 0), stop=(k2 == DC // 2 - 1),
                            perf_mode=DR)
                    if no % 2 == 0:
                        nc.scalar.activation(
                            ktch[:, no * T:(no + 1) * T], ps,
                            AF.Identity, bias=bk_pc[:, hp:hp + 1])
                    else:
                        nc.vector.tensor_scalar_add(
                            ktch[:, no * T:(no + 1) * T], ps,
                            bk_pc[:, hp:hp + 1])
                return ktch

            def attn_scores(b, hp, so2, ktch, ps_ctx, psSC):
                ex = expp.tile([P, 2, 2, 512], fp8, tag="exp", name=f"ex{b}")
                for j in range(2):
                    so = so2 + j
                    scj = [psSC.tile([P, 512], f32, tag="psSC",
                                     name=f"sc{b}{j}{hh}") for hh in range(2)]
                    for hh in range(2):
                        base = hh * HD
                        nc.tensor.matmul(
                            scj[hh][:],
                            ktch[base:base + HD, so * P:(so + 1) * P],
                            qt8[b][base:base + HD, hp, :],
                            start=True, stop=True)
                    # exp: ACT handles j0 chunks, DVE (schraudolph) j1
                    for hh in range(2):
                        if j == 0:
                            nc.scalar.activation(ex[:, hh, 0, :], scj[hh][:],
                                                 AF.Exp, scale=1.0 / SPS)
                        else:
                            nc.vector.tensor_scalar(
                                ex[:, hh, 1, :].bitcast(u8), scj[hh][:],
                                EXPA, EXPC, op0=ALU.mult, op1=ALU.add)
                for hh in range(2):
                    h = 2 * hp + hh
                    nc.tensor.matmul(
                        ps_ctx[hh][:], v8[b][:, so2:so2 + 2, h, :],
                        ex[:, hh, :, :], start=(so2 == 0),
                        stop=(so2 == SC - 2), perf_mode=DR)

            def attn_evict(b, hp, hh, ps_ctx, psSC):
                base = hh * HD
                rden_f = evp.tile([1, T], f32, tag="rden_f")
                nc.scalar.mul(rden_f[:], ps_ctx[hh][HD:HD + 1, :], 16.0)
                rrec = evp.tile([1, T], f32, tag="rrec")
                nc.vector.reciprocal_approx_fast(out=rrec[:], in_=rden_f[:])
                rden_bf = evp.tile([1, T], bf16, tag="rden_bf")
                nc.scalar.copy(rden_bf[:], rrec[:])
                ps_bct = psSC.tile([P, 512], f32, tag="psSC")
                ps_bc = ps_bct[:]
                nc.tensor.matmul(ps_bc[0:HD, :], ones64_bf[:],
                                 rden_bf[:], start=True, stop=True)
                bc_sb = evp.tile([HD, T], bf16, tag="bc_sb")
                nc.scalar.copy(bc_sb[:], ps_bc[0:HD, :])
                nc.vector.tensor_tensor(
                    ctxT[b][base:base + HD, hp, :],
                    ps_ctx[hh][0:HD, :], bc_sb[:], ALU.mult)

            with (
                tc.tile_pool(name="psSC", bufs=4, space="PSUM") as psSC,
                tc.tile_pool(name="psCTX", bufs=4, space="PSUM") as psCTX,
            ):
                for hp in range(DC):
                    ktch = [attn_kproj(b, hp, psSC) for b in range(BPC)]
                    ps_ctx = [[psCTX.tile([VPAD, T], f32, tag="psCTX",
                                          name=f"ps_ctx{b}_{i}")
                               for i in range(2)] for b in range(BPC)]
                    for so2 in range(0, SC, 2):
                        for b in range(BPC):
                            attn_scores(b, hp, so2, ktch[b], ps_ctx[b], psSC)
                    for b in range(BPC):
                        for hh in range(2):
                            attn_evict(b, hp, hh, ps_ctx[b], psSC)

            # =========== per-batch tail helpers ===========

            def phase_C(b, xT, psP):
                for mo in range(DC):
                    ps = psP.tile([P, 512], f32, tag="psP")
                    for ki in range(DC):
                        nc.tensor.matmul(ps[:], wo_sb[:, ki, mo * P:(mo + 1) * P],
                                         ctxT[b][:, ki, :],
                                         start=(ki == 0), stop=(ki == DC - 1))
                    nc.vector.scalar_tensor_tensor(
                        xT[:, mo, :], ps[:], bo_pc[:, mo:mo + 1],
                        qbf[b][:, mo, :], op0=ALU.add, op1=ALU.add)

            def ln_pass(b, tag, xsrc, g_row, bn_row, dst=None, store=False):
                with (
                    tc.tile_pool(name=f"psST{tag}{b}", bufs=1,
                                 space="PSUM") as ps_st,
                    tc.tile_pool(name=f"psLB{tag}{b}", bufs=2,
                                 space="PSUM") as ps_bc,
                ):
                    psum_mu = ps_st.tile([1, T], f32, tag="st_mu")
                    psum_sq = ps_st.tile([1, T], f32, tag="st_sq")
                    for c in range(DC):
                        nc.tensor.matmul(psum_mu[:], ones_col_bf[:],
                                         xsrc[:, c, :],
                                         start=(c == 0), stop=(c == DC - 1))
                    sqt = []
                    for c in range(DC):
                        sq = tmpp.tile([P, T], bf16, tag="lnsq")
                        nc.vector.tensor_mul(sq[:], xsrc[:, c, :], xsrc[:, c, :])
                        sqt.append(sq)
                    for c in range(DC):
                        nc.tensor.matmul(psum_sq[:], ones_col_bf[:], sqt[c][:],
                                         start=(c == 0), stop=(c == DC - 1))
                    mu_f = statp.tile([1, T], f32, tag="ln_mu")
                    nc.vector.tensor_scalar_mul(mu_f[:], psum_mu[:], 1.0 / D)
                    mu2_f = statp.tile([1, T], f32, tag="ln_mu2")
                    nc.vector.tensor_tensor(mu2_f[:], mu_f[:], mu_f[:], ALU.mult)
                    var_f = statp.tile([1, T], f32, tag="ln_var")
                    nc.vector.scalar_tensor_tensor(
                        var_f[:], psum_sq[:], 1.0 / D, mu2_f[:],
                        op0=ALU.mult, op1=ALU.subtract)
                    rs_f = statp.tile([1, T], f32, tag="ln_rs")
                    nc.scalar.activation(rs_f[:], var_f[:],
                                         AF.Abs_reciprocal_sqrt, bias=eps_t[:])
                    rs_r = statp.tile([1, T], f32r, tag="ln_rs_r")
                    nc.vector.tensor_copy(rs_r[:], rs_f[:])
                    mrs_r = statp.tile([1, T], f32r, tag="ln_mrs_r")
                    nc.vector.tensor_tensor(mrs_r[:], mu_f[:], rs_f[:],
                                            ALU.mult)
                    for c in range(DC):
                        bcA = ps_bc.tile([P, T], f32, tag="ln_bcA")
                        bcB = ps_bc.tile([P, T], f32, tag="ln_bcB")
                        gsl = g_row[:, c * P:(c + 1) * P]
                        bsl = bn_row[:, c * P:(c + 1) * P]
                        nc.tensor.matmul(bcA[:], gsl, rs_r[:],
                                         start=True, stop=True)
                        nc.tensor.matmul(bcB[:], gsl, mrs_r[:],
                                         start=True, stop=False)
                        nc.tensor.matmul(bcB[:], bsl, ones_row[:],
                                         start=False, stop=True)
                        tmp = tmpp.tile([P, T], f32, tag="ln_tmp")
                        nc.vector.tensor_tensor(tmp[:], xsrc[:, c, :], bcA[:],
                                                ALU.mult)
                        if store:
                            oc = outp.tile([P, T], f32, tag="outT")
                            nc.vector.tensor_tensor(oc[:], tmp[:], bcB[:],
                                                    ALU.subtract)
                            nc.sync.dma_start(
                                out_d.ap()[b][c * P:(c + 1) * P, :], oc[:])
                        else:
                            nc.vector.tensor_tensor(dst[:, c, :], tmp[:],
                                                    bcB[:], ALU.subtract)

            def phase_E(b, xT, hT, x2T):
                with (
                    tc.tile_pool(name=f"psE1{b}", bufs=2, space="PSUM") as psE1,
                    tc.tile_pool(name=f"psF2{b}", bufs=6, space="PSUM") as psF2,
                ):
                    ps_f2 = [psF2.tile([P, T], f32, tag="psF2", name=f"psf2_{i}")
                             for i in range(DC)]
                    for g in range(FOG):
                        f1g = fstream.tile([P, DC, FPG * P], bf16, tag="f1g")
                        nc.sync.dma_start(f1g[:], fc1_d.ap().rearrange(
                            "(k p) f -> p k f", p=P)[:, :, g * FPG * P:
                                                     (g + 1) * FPG * P])
                        f2g = fstream.tile([P, FPG, D], bf16, tag="f2g")
                        nc.gpsimd.dma_start(f2g[:], fc2_d.ap().rearrange(
                            "(ko p) o -> p ko o", p=P)[:, g * FPG:(g + 1) * FPG, :])
                        for j in range(FPG):
                            fo = g * FPG + j
                            ps1 = psE1.tile([P, 512], f32, tag="psE1")
                            for ki in range(DC):
                                nc.tensor.matmul(
                                    ps1[:], f1g[:, ki, j * P:(j + 1) * P],
                                    hT[:, ki, :],
                                    start=(ki == 0), stop=(ki == DC - 1))
                            mch = mchunkp.tile([P, T], bf16, tag="mch")
                            nc.scalar.activation(mch[:], ps1[:], AF.Gelu,
                                                 bias=f1b_pc[:, fo:fo + 1])
                            for mo in range(DC):
                                nc.tensor.matmul(
                                    ps_f2[mo][:], f2g[:, j, mo * P:(mo + 1) * P],
                                    mch[:],
                                    start=(fo == 0), stop=(fo == FFC - 1))
                    for mo in range(DC):
                        nc.vector.scalar_tensor_tensor(
                            x2T[:, mo, :], ps_f2[mo][:], f2b_pc[:, mo:mo + 1],
                            xT[:, mo, :], op0=ALU.add, op1=ALU.add)

            # =========== tail emission ===========
            xT = [None, None]
            hT = [None, None]
            x2T = [None, None]
            with tc.tile_pool(name="psC", bufs=2, space="PSUM") as psP2:
                for b in range(BPC):
                    xT[b] = xtp.tile([P, DC, T], bf16, tag="xT", name=f"xT{b}")
                    phase_C(b, xT[b], psP2)
            hT[0] = htp.tile([P, DC, T], bf16, tag="hT", name="hT0")
            ln_pass(0, "2", xT[0], ln2g, ln2bn, dst=hT[0])
            x2T[0] = x2tp.tile([P, DC, T], bf16, tag="x2T", name="x2T0")
            phase_E(0, xT[0], hT[0], x2T[0])
            hT[1] = htp.tile([P, DC, T], bf16, tag="hT", name="hT1")
            ln_pass(1, "2", xT[1], ln2g, ln2bn, dst=hT[1])
            ln_pass(0, "p", x2T[0], lnpg, lnpbn, store=True)
            x2T[1] = x2tp.tile([P, DC, T], bf16, tag="x2T", name="x2T1")
            phase_E(1, xT[1], hT[1], x2T[1])
            ln_pass(1, "p", x2T[1], lnpg, lnpbn, store=True)

    nc.compile()
    return nc


def _get_nc(use_bv: bool):
    key = ("nc", use_bv)
    if key not in _cached:
        _cached[key] = _build(use_bv)
    return _cached[key]


def kernel(query, key, wq, bq, wk, bk, wv, bv, wo, bo,
           ln2_g, ln2_b, fc1_w, fc1_b, fc2_w, fc2_b, lnp_g, lnp_b):
    import ml_dtypes
    from concourse.bass_utils import run_bass_kernel_spmd

    f = np.float32
    bf = ml_dtypes.bfloat16
    f8 = ml_dtypes.float8_e4m3fn
    c = np.ascontiguousarray
    query = np.asarray(query, f)
    key = np.asarray(key, f)
    use_bv = bool(np.any(np.asarray(bv)))
    nc = _get_nc(use_bv)

    shared = {
        "wq_t": c(np.asarray(wq, f).T.astype(bf)),
        "wk8": c((np.asarray(wk, f).T * np.float32(KSC)).astype(f8)),
        "wv8": c((np.asarray(wv, f).T * np.float32(KSC)).astype(f8)),
        "wo_t": c(np.asarray(wo, f).T.astype(bf)),
        "fc1_t": c(np.asarray(fc1_w, f).T.astype(bf)),
        "fc2_t": c(np.asarray(fc2_w, f).T.astype(bf)),
        "bqv": c(np.asarray(bq, f)),
        "bk64": c(np.asarray(bk, f) * np.float32(KSC)),
        "bv16": c(np.asarray(bv, f).reshape(1, D) * np.float32(16.0)),
        "bov": c(np.asarray(bo, f)),
        "f1b": c(np.asarray(fc1_b, f)),
        "f2b": c(np.asarray(fc2_b, f)),
        "ln2g": c(np.asarray(ln2_g, f).reshape(1, D)),
        "ln2bn": c(-np.asarray(ln2_b, f).reshape(1, D)),
        "lnpg": c(np.asarray(lnp_g, f).reshape(1, D)),
        "lnpbn": c(-np.asarray(lnp_b, f).reshape(1, D)),
        "ones_row": np.ones((1, T), f),
    }
    in_maps = []
    for core in range(NCORES):
        sl = slice(core * BPC, (core + 1) * BPC)
        m = dict(shared)
        m["qbf"] = c(query[sl].transpose(0, 2, 1).astype(bf))
        m["k8"] = c(key[sl].transpose(0, 2, 1).astype(f8))
        in_maps.append(m)

    res = run_bass_kernel_spmd(nc, in_maps, core_ids=list(range(NCORES)))
    kernel._last_result = res
    out = np.concatenate([r["out"] for r in res.results], axis=0)
    return c(out.transpose(0, 2, 1))
